# revision 28
# baseline (speedup 1.0000x reference)
"""Trainium2 Bass kernel for nn_AdaptiveNoiseScheduler (segment_reduce).

Distribution: 8 NeuronCores = 4 batches x 2 sequence-halves, 2048 tokens/core,
MLP weights replicated (host pre-cast to bf16). The context term is rewritten
so the sequence cumsum happens AFTER the W1b projection (cumsum commutes with
the matmul), letting each core run fp32 free-axis DVE scans over its projected
activations. Cross-shard scan carries are 1024-float vectors: host folds them
into the scan initial state (bwd carry) and a rank-1 PSUM term (whole-batch
total), so no collectives are needed.

Layer 2 runs in fp8-e4m3 DoubleRow (weights host-prescaled by 32, compensated
via the gelu2 activation scale); layer 1 and the scan stay bf16/fp32.

Per-token pre-activation of layer 1:
    pre1[t] = h[t] @ W1a + b1 + A_t * U_tot + (B_t - A_t) * cu[t] - B_t * u[t]
where u = h @ W1b, cu = carry + inclusive-cumsum(u), A_t = 0.5/(S-1-t) gated,
B_t = 0.5/t gated. Everything runs in a transposed (feature-partition,
token-free) layout; biases ride the scalar-engine activation bias port; the
tiny elementwise finish (timestep scale, mask boost, clip, int cast) runs on
host over the (4,4096) result.
"""

from contextlib import ExitStack

import numpy as np
import ml_dtypes

P = 128
B, S, E = 4, 4096, 1024
T = S // 2          # tokens per core
TH = 1024           # tokens per PSUM stage (2 per core)
F1, F2 = 1024, 512
NE, NF1, NF2 = E // P, F1 // P, F2 // P
NUM_TIMESTEPS = 1000

_COMPILED = None


def _build_nc(act="Gelu"):
    import concourse.mybir as mybir
    import concourse.tile as tile
    from concourse import bacc
    from concourse.masks import make_identity

    f32, bf16 = mybir.dt.float32, mybir.dt.bfloat16
    AF = mybir.ActivationFunctionType
    ACT_FN = getattr(AF, act)
    ADD = mybir.AluOpType.add

    nc = bacc.Bacc("TRN2", target_bir_lowering=False, debug=False, num_devices=8)

    h_d = nc.dram_tensor("h", (T, E), bf16, kind="ExternalInput").ap()
    w1_d = nc.dram_tensor("w1", (2 * E, F1), bf16, kind="ExternalInput").ap()
    w2_d = nc.dram_tensor("w2", (F1, F2), mybir.dt.float8e4, kind="ExternalInput").ap()
    w3_d = nc.dram_tensor("w3r", (P, NF2), bf16, kind="ExternalInput").ap()
    uc_d = nc.dram_tensor("ucol", (P, NF1), f32, kind="ExternalInput").ap()
    b1c_d = nc.dram_tensor("b1c", (P, NF1), f32, kind="ExternalInput").ap()
    b2c_d = nc.dram_tensor("b2c", (P, NF2), f32, kind="ExternalInput").ap()
    car_d = nc.dram_tensor("carry", (P, NF1), f32, kind="ExternalInput").ap()
    bab_d = nc.dram_tensor("bab", (P, T), bf16, kind="ExternalInput").ap()
    bb_d = nc.dram_tensor("bb", (P, T), bf16, kind="ExternalInput").ap()
    out_d = nc.dram_tensor("out", (T,), f32, kind="ExternalOutput").ap()

    with tile.TileContext(nc) as tc, ExitStack() as ctx:
        const = ctx.enter_context(tc.tile_pool(name="const", bufs=1))
        hstage = ctx.enter_context(tc.tile_pool(name="hstage", bufs=2))
        big = ctx.enter_context(tc.tile_pool(name="big", bufs=1))
        cupool = ctx.enter_context(tc.tile_pool(name="cupool", bufs=3))
        tmp = ctx.enter_context(tc.tile_pool(name="tmp", bufs=2))
        psA = ctx.enter_context(tc.tile_pool(name="psA", bufs=4, space="PSUM"))
        psB = psA

        # ---- identity for PE transposes, built on the (idle) gpsimd engine
        ident = const.tile([P, P], bf16, name="ident")
        make_identity(nc, ident[:])

        # ---- h loads (HWDGE bf16), interleaved with W1 by first use ----
        hrows = [None] * 16
        w1_t = [None] * 16

        def load_hrow(r):
            t = hstage.tile([P, E], bf16, tag="hrow", name=f"hrow_{r}", bufs=16)
            nc.sync.dma_start(t[:], h_d[r * P:(r + 1) * P, :])
            hrows[r] = t

        def load_w1(r):
            t = const.tile([P, F1], bf16, name=f"w1_{r}")
            nc.sync.dma_start(t[:], w1_d[r * P:(r + 1) * P, :])
            w1_t[r] = t

        for r in range(8):          # first token half of h
            load_hrow(r)
        for r in range(8, 16):      # W1b (u-matmuls come first)
            load_w1(r)

        zeros = const.tile([P, TH], bf16, name="zeros")
        nc.gpsimd.memset(zeros[:], 0.0)

        # ---- aux: tiny tensors first, coefficient rows split per token half
        ucol = const.tile([P, NF1], f32, name="ucol_t")
        nc.sync.dma_start(ucol[:], uc_d[:, :])
        carry = const.tile([P, NF1], f32, name="carry_t")
        nc.sync.dma_start(carry[:], car_d[:, :])
        b1c = const.tile([P, NF1], f32, name="b1c")
        nc.sync.dma_start(b1c[:], b1c_d[:, :])
        w3_t = const.tile([P, NF2], bf16, name="w3_t")
        nc.sync.dma_start(w3_t[:], w3_d[:, :])
        b2c = const.tile([P, NF2], f32, name="b2c")
        nc.sync.dma_start(b2c[:], b2c_d[:, :])
        bab = const.tile([P, T], bf16, name="bab_t")
        bb = const.tile([P, T], bf16, name="bb_t")
        a_bc = const.tile([P, T], bf16, name="a_bc")
        nc.sync.dma_start(bab[:, :TH], bab_d[:, :TH])
        nc.sync.dma_start(bb[:, :TH], bb_d[:, :TH])
        nc.vector.tensor_sub(a_bc[:, :TH], bb[:, :TH], bab[:, :TH])
        for r in range(8):          # W1a (needed from the first fc's psV)
            load_w1(r)
        for r in range(8, 16):      # second token half of h (needed ~70us in)
            load_hrow(r)
        nc.sync.dma_start(bab[:, TH:], bab_d[:, TH:])
        nc.sync.dma_start(bb[:, TH:], bb_d[:, TH:])
        nc.vector.tensor_sub(a_bc[:, TH:], bb[:, TH:], bab[:, TH:])
        w2_t = []
        for r in range(NF1 // 2):
            t = const.tile([P, 2, F2], mybir.dt.float8e4, name=f"w2_{r}")
            nc.sync.dma_start(
                t[:],
                w2_d[2 * r * P:(2 * r + 2) * P, :].rearrange("(g p) f -> p g f", p=P),
            )
            w2_t.append(t)

        ht = big.tile([P, NE, T], bf16, name="ht")
        x1 = big.tile([P, NF1, T], mybir.dt.float8e4, name="x1")
        x2 = big.tile([P, NF2, T], bf16, name="x2")
        nf = big.tile([1, T], f32, name="nf")
        cu_last = big.tile([P, NF1], bf16, name="cu_last")

        # ---- phase 0: PE-transpose h rows into ht, two rows per PSUM tile.
        #      Upfront copies ride DVE (idle then); in-sweep ones ride ACT.
        def transpose_rows(r0, eng="vector"):
            pst = psA.tile([P, 2 * TH], bf16, tag="psA", name=f"pst_{r0}")
            for g in range(2):
                for ec in range(NE):
                    nc.tensor.transpose(
                        pst[:, g * TH + ec * P:g * TH + (ec + 1) * P],
                        hrows[r0 + g][:, ec * P:(ec + 1) * P],
                        ident[:],
                    )
            dst = ht[:, :, r0 * P:(r0 + 2) * P].rearrange("p a (g b) -> p g a b", g=2)
            src = pst[:].rearrange("p (g a b) -> p g a b", g=2, b=P)
            if eng == "vector":
                nc.vector.tensor_copy(dst, src)
            else:
                nc.scalar.copy(dst, src)

        for r0 in range(0, 8, 2):
            transpose_rows(r0)

        # ---- phase 1 (th-major: first token half across all fc, then second;
        #      rows 8-15 transposes slot in during the th0 sweep)
        for th in range(2):
            t0 = th * TH
            for fc in range(NF1):
                if th == 0 and fc in (2, 3, 4, 5):
                    transpose_rows(8 + 2 * (fc - 2), eng="scalar")
                psU = psA.tile([P, TH], f32, tag="psA", name=f"psU_{fc}_{th}")
                psV = psB.tile([P, TH], f32, tag="psA", name=f"psV_{fc}_{th}")
                for q in range(2):
                    sl = slice(t0 + q * 512, t0 + (q + 1) * 512)
                    qsl = slice(q * 512, (q + 1) * 512)
                    for ec in range(NE):
                        nc.tensor.matmul(
                            psU[:, qsl], w1_t[8 + ec][:, fc * P:(fc + 1) * P],
                            ht[:, ec, sl], start=(ec == 0), stop=(ec == NE - 1),
                        )
                for q in range(2):
                    sl = slice(t0 + q * 512, t0 + (q + 1) * 512)
                    qsl = slice(q * 512, (q + 1) * 512)
                    for ec in range(NE):
                        nc.tensor.matmul(
                            psV[:, qsl], w1_t[ec][:, fc * P:(fc + 1) * P],
                            ht[:, ec, sl], start=(ec == 0), stop=(ec == NE - 1),
                        )
                cu = cupool.tile([P, TH], mybir.dt.bfloat16, tag="cu",
                                 name=f"cu_{fc}_{th}")
                init = carry[:, fc:fc + 1] if th == 0 else cu_last[:, fc:fc + 1]
                nc.vector.tensor_tensor_scan(
                    cu[:], psU[:], zeros[:], init, op0=ADD, op1=ADD
                )
                if th == 0:
                    nc.vector.tensor_copy(cu_last[:, fc:fc + 1], cu[:, TH - 1:TH])
                sl = slice(t0, t0 + TH)
                t1 = tmp.tile([P, TH], mybir.dt.bfloat16, tag="t1", name="t1")
                nc.vector.tensor_mul(t1[:], cu[:], bab[:, sl])
                wv = tmp.tile([P, TH], mybir.dt.bfloat16, tag="wv", name="wv")
                nc.vector.tensor_mul(wv[:], psU[:], bb[:, sl])
                z2 = tmp.tile([P, TH], mybir.dt.bfloat16, tag="z2", name="z2")
                nc.vector.scalar_tensor_tensor(
                    z2[:], a_bc[:, sl], ucol[:, fc:fc + 1], t1[:],
                    op0=mybir.AluOpType.mult, op1=mybir.AluOpType.add,
                )
                zv = tmp.tile([P, TH], mybir.dt.bfloat16, tag="zv", name="zv")
                nc.vector.tensor_add(zv[:], psV[:], z2[:])
                p1 = tmp.tile([P, TH], mybir.dt.bfloat16, tag="p1", name="p1")
                nc.vector.tensor_sub(p1[:], zv[:], wv[:])
                nc.scalar.activation(
                    x1[:, fc, sl], p1[:], ACT_FN, bias=b1c[:, fc:fc + 1]
                )

        # ---- phase 2+3 per token half ----
        for th in range(2):
            t0 = th * TH
            for fc2 in range(NF2):
                psX = psA.tile([P, TH], f32, tag="psA", name=f"psX_{fc2}_{th}")
                for q in range(2):
                    sl = slice(t0 + q * 512, t0 + (q + 1) * 512)
                    qsl = slice(q * 512, (q + 1) * 512)
                    for r2 in range(NF1 // 2):
                        nc.tensor.matmul(
                            psX[:, qsl],
                            w2_t[r2][:, :, fc2 * P:(fc2 + 1) * P],
                            x1[:, 2 * r2:2 * r2 + 2, sl],
                            start=(r2 == 0), stop=(r2 == NF1 // 2 - 1),
                            perf_mode=mybir.MatmulPerfMode.DoubleRow,
                        )
                nc.scalar.activation(
                    x2[:, fc2, t0:t0 + TH], psX[:], ACT_FN,
                    bias=b2c[:, fc2:fc2 + 1], scale=float(1.0 / 32.0),
                )
            for q in range(2):
                sl = slice(t0 + q * 512, t0 + (q + 1) * 512)
                psN = psB.tile([1, 512], f32, tag="psA", name=f"psN_{th}_{q}")
                for kc in range(NF2):
                    nc.tensor.matmul(
                        psN[:], w3_t[:, kc:kc + 1], x2[:, kc, sl],
                        start=(kc == 0), stop=(kc == NF2 - 1),
                    )
                nc.vector.tensor_copy(nf[:, sl], psN[:])
                nc.sync.dma_start(
                    out_d.rearrange("(a b) -> a b", a=1)[:, sl], nf[:, sl]
                )

    nc.compile()
    return nc


def _get_compiled():
    global _COMPILED
    if _COMPILED is None:
        _COMPILED = _build_nc()
    return _COMPILED


def _make_in_maps(inputs):
    bf = ml_dtypes.bfloat16
    h = np.ascontiguousarray(np.asarray(inputs["hidden_states"], dtype=np.float32))
    W1 = np.asarray(inputs["W1"], dtype=np.float32)
    W2 = np.asarray(inputs["W2"], dtype=np.float32)
    W3 = np.asarray(inputs["W3"], dtype=np.float32)
    b1 = np.asarray(inputs["b1"], dtype=np.float32)
    b2 = np.asarray(inputs["b2"], dtype=np.float32)
    b3 = np.asarray(inputs["b3"], dtype=np.float32)

    i = np.arange(S, dtype=np.float64)
    A = np.where(i < S - 1, 0.5 / np.maximum(S - 1 - i, 1), 0.0).astype(np.float32)
    Bv = np.where(i > 0, 0.5 / np.maximum(i, 1), 0.0).astype(np.float32)
    BA = (Bv - A).astype(np.float32)

    W1b = W1[E:]
    w1_bf = np.ascontiguousarray(W1.astype(bf))
    w2_f8 = np.ascontiguousarray((W2 * np.float32(32.0)).astype(ml_dtypes.float8_e4m3))
    w3r = np.ascontiguousarray(W3[:, 0].reshape(NF2, P).T.astype(bf))
    b1c = np.ascontiguousarray(b1.reshape(NF1, P).T)
    b2c = np.ascontiguousarray(b2.reshape(NF2, P).T)

    in_maps = []
    for c in range(8):
        bi, half = divmod(c, 2)
        sl = slice(half * T, (half + 1) * T)
        U = (h[bi].sum(0, dtype=np.float64) @ W1b.astype(np.float64)).astype(np.float32)
        if half == 0:
            Bc = np.zeros(E, np.float32)
        else:
            Bc = (h[bi, :T].sum(0, dtype=np.float64) @ W1b.astype(np.float64)).astype(np.float32)
        in_maps.append({
            "h": np.ascontiguousarray(h[bi, sl].astype(bf)),
            "w1": w1_bf,
            "w2": w2_f8,
            "w3r": w3r,
            "ucol": np.ascontiguousarray(U.reshape(NF1, P).T),
            "b1c": b1c,
            "b2c": b2c,
            "carry": np.ascontiguousarray(Bc.reshape(NF1, P).T),
            "bab": np.ascontiguousarray(np.broadcast_to(BA[sl].astype(bf), (P, T))),
            "bb": np.ascontiguousarray(np.broadcast_to(Bv[sl].astype(bf), (P, T))),
        })
    return in_maps


def _finish(logits, inputs):
    b3 = np.asarray(inputs["b3"], dtype=np.float32)
    nf = np.float32(1.0) / (np.float32(1.0) + np.exp(-(logits + b3[0])))
    gt = np.float32(np.asarray(inputs["global_timestep"]))
    mask = np.asarray(inputs["token_mask"])
    ad = gt * (np.float32(0.5) + nf.astype(np.float32))
    ad = ad * (np.float32(1.0) + mask.astype(np.float32) * np.float32(0.3))
    ad = np.clip(ad, np.float32(0.0), np.float32(NUM_TIMESTEPS - 1))
    return ad.astype(np.int32)


def kernel(**inputs):
    from concourse import bass_utils

    nc = _get_compiled()
    in_maps = _make_in_maps(inputs)
    res = bass_utils.run_bass_kernel_spmd(nc, in_maps, core_ids=list(range(8)))
    nf = np.zeros((B, S), np.float32)
    for c in range(8):
        bi, half = divmod(c, 2)
        nf[bi, half * T:(half + 1) * T] = res.results[c]["out"]
    return _finish(nf, inputs)


# revision 31
# speedup vs baseline: 1.1651x; 1.1651x over previous
"""Trainium2 Bass kernel for nn_AdaptiveNoiseScheduler (segment_reduce).

Distribution: 8 NeuronCores = 4 batches x 2 sequence-halves, 2048 tokens/core,
MLP weights replicated (host pre-cast to bf16). The context term is rewritten
so the sequence cumsum happens AFTER the W1b projection (cumsum commutes with
the matmul), letting each core run fp32 free-axis DVE scans over its projected
activations. Cross-shard scan carries are 1024-float vectors: host folds them
into the scan initial state (bwd carry) and a rank-1 PSUM term (whole-batch
total), so no collectives are needed.

Layer 2 runs in fp8-e4m3 DoubleRow (weights host-prescaled by 32, compensated
via the gelu2 activation scale); layer 1 and the scan stay bf16/fp32.

Per-token pre-activation of layer 1:
    pre1[t] = h[t] @ W1a + b1 + A_t * U_tot + (B_t - A_t) * cu[t] - B_t * u[t]
where u = h @ W1b, cu = carry + inclusive-cumsum(u), A_t = 0.5/(S-1-t) gated,
B_t = 0.5/t gated. Everything runs in a transposed (feature-partition,
token-free) layout; biases ride the scalar-engine activation bias port; the
tiny elementwise finish (timestep scale, mask boost, clip, int cast) runs on
host over the (4,4096) result.
"""

from contextlib import ExitStack

import numpy as np
import ml_dtypes

P = 128
B, S, E = 4, 4096, 1024
T = S // 2          # tokens per core
TH = 1024           # tokens per PSUM stage (2 per core)
F1, F2 = 1024, 512
NE, NF1, NF2 = E // P, F1 // P, F2 // P
NUM_TIMESTEPS = 1000

_COMPILED = None


def _build_nc(act="Gelu"):
    import concourse.mybir as mybir
    import concourse.tile as tile
    from concourse import bacc
    from concourse.masks import make_identity

    f32, bf16 = mybir.dt.float32, mybir.dt.bfloat16
    AF = mybir.ActivationFunctionType
    ACT_FN = getattr(AF, act)
    ADD = mybir.AluOpType.add

    nc = bacc.Bacc("TRN2", target_bir_lowering=False, debug=False, num_devices=8)

    h_d = nc.dram_tensor("h", (T, E), bf16, kind="ExternalInput").ap()
    w1_d = nc.dram_tensor("w1", (2 * E, F1), mybir.dt.float8e4, kind="ExternalInput").ap()
    w2_d = nc.dram_tensor("w2", (F1, F2), mybir.dt.float8e4, kind="ExternalInput").ap()
    w3_d = nc.dram_tensor("w3r", (P, NF2), bf16, kind="ExternalInput").ap()
    uc_d = nc.dram_tensor("ucol", (P, NF1), f32, kind="ExternalInput").ap()
    b1c_d = nc.dram_tensor("b1c", (P, NF1), f32, kind="ExternalInput").ap()
    b2c_d = nc.dram_tensor("b2c", (P, NF2), f32, kind="ExternalInput").ap()
    car_d = nc.dram_tensor("carry", (P, NF1), f32, kind="ExternalInput").ap()
    bab_d = nc.dram_tensor("bab", (P, T), bf16, kind="ExternalInput").ap()
    bb_d = nc.dram_tensor("bb", (P, T), bf16, kind="ExternalInput").ap()
    out_d = nc.dram_tensor("out", (T,), f32, kind="ExternalOutput").ap()

    with tile.TileContext(nc) as tc, ExitStack() as ctx:
        const = ctx.enter_context(tc.tile_pool(name="const", bufs=1))
        hstage = ctx.enter_context(tc.tile_pool(name="hstage", bufs=2))
        big = ctx.enter_context(tc.tile_pool(name="big", bufs=1))
        cupool = ctx.enter_context(tc.tile_pool(name="cupool", bufs=3))
        tmp = ctx.enter_context(tc.tile_pool(name="tmp", bufs=2))
        psA = ctx.enter_context(tc.tile_pool(name="psA", bufs=4, space="PSUM"))
        psB = psA

        # ---- identity for PE transposes, built on the (idle) gpsimd engine
        ident = const.tile([P, P], bf16, name="ident")
        make_identity(nc, ident[:])

        # ---- h loads (HWDGE bf16), interleaved with W1 by first use ----
        hrows = [None] * 16
        w1_t = [None] * 8

        def load_hrow(r):
            t = hstage.tile([P, E], bf16, tag="hrow", name=f"hrow_{r}", bufs=16)
            nc.sync.dma_start(t[:], h_d[r * P:(r + 1) * P, :])
            hrows[r] = t

        def load_w1(r2):
            t = const.tile([P, 2, F1], mybir.dt.float8e4, name=f"w1_{r2}")
            nc.sync.dma_start(
                t[:],
                w1_d[2 * r2 * P:(2 * r2 + 2) * P, :].rearrange(
                    "(g p) f -> p g f", p=P
                ),
            )
            w1_t[r2] = t

        for r in range(8):          # first token half of h
            load_hrow(r)
        for r2 in range(4, 8):      # W1b (u-matmuls come first)
            load_w1(r2)

        zeros = const.tile([P, TH], bf16, name="zeros")
        nc.gpsimd.memset(zeros[:], 0.0)

        # ---- aux: tiny tensors first, coefficient rows split per token half
        ucol = const.tile([P, NF1], f32, name="ucol_t")
        nc.sync.dma_start(ucol[:], uc_d[:, :])
        carry = const.tile([P, NF1], f32, name="carry_t")
        nc.sync.dma_start(carry[:], car_d[:, :])
        b1c = const.tile([P, NF1], f32, name="b1c")
        nc.sync.dma_start(b1c[:], b1c_d[:, :])
        w3_t = const.tile([P, NF2], bf16, name="w3_t")
        nc.sync.dma_start(w3_t[:], w3_d[:, :])
        b2c = const.tile([P, NF2], f32, name="b2c")
        nc.sync.dma_start(b2c[:], b2c_d[:, :])
        bab = const.tile([P, T], bf16, name="bab_t")
        bb = const.tile([P, T], bf16, name="bb_t")
        a_bc = const.tile([P, T], bf16, name="a_bc")
        nc.sync.dma_start(bab[:, :TH], bab_d[:, :TH])
        nc.sync.dma_start(bb[:, :TH], bb_d[:, :TH])
        nc.vector.tensor_sub(a_bc[:, :TH], bb[:, :TH], bab[:, :TH])
        for r2 in range(4):         # W1a (needed from the first fc's psV)
            load_w1(r2)
        for r in range(8, 16):      # second token half of h (needed ~70us in)
            load_hrow(r)
        nc.sync.dma_start(bab[:, TH:], bab_d[:, TH:])
        nc.sync.dma_start(bb[:, TH:], bb_d[:, TH:])
        nc.vector.tensor_sub(a_bc[:, TH:], bb[:, TH:], bab[:, TH:])
        w2_t = []
        for r in range(NF1 // 2):
            t = const.tile([P, 2, F2], mybir.dt.float8e4, name=f"w2_{r}")
            nc.sync.dma_start(
                t[:],
                w2_d[2 * r * P:(2 * r + 2) * P, :].rearrange("(g p) f -> p g f", p=P),
            )
            w2_t.append(t)

        ht = big.tile([P, NE, T], mybir.dt.float8e4, name="ht")
        x1 = big.tile([P, NF1, T], mybir.dt.float8e4, name="x1")
        x2 = big.tile([P, NF2, T], bf16, name="x2")
        nf = big.tile([1, T], f32, name="nf")
        cu_last = big.tile([P, NF1], bf16, name="cu_last")

        # ---- phase 0: PE-transpose h rows into ht, two rows per PSUM tile.
        #      Upfront copies ride DVE (idle then); in-sweep ones ride ACT.
        def transpose_rows(r0, eng="vector"):
            pst = psA.tile([P, 2 * TH], bf16, tag="psA", name=f"pst_{r0}")
            for g in range(2):
                for ec in range(NE):
                    nc.tensor.transpose(
                        pst[:, g * TH + ec * P:g * TH + (ec + 1) * P],
                        hrows[r0 + g][:, ec * P:(ec + 1) * P],
                        ident[:],
                    )
            dst = ht[:, :, r0 * P:(r0 + 2) * P].rearrange("p a (g b) -> p g a b", g=2)
            src = pst[:].rearrange("p (g a b) -> p g a b", g=2, b=P)
            if eng == "vector":
                nc.vector.tensor_copy(dst, src)
            else:
                nc.scalar.copy(dst, src)

        for r0 in range(0, 8, 2):
            transpose_rows(r0)

        # ---- phase 1 (th-major: first token half across all fc, then second;
        #      rows 8-15 transposes slot in during the th0 sweep)
        for th in range(2):
            t0 = th * TH
            for fc in range(NF1):
                if th == 0 and fc in (2, 3, 4, 5):
                    transpose_rows(8 + 2 * (fc - 2), eng="scalar")
                psU = psA.tile([P, TH], f32, tag="psA", name=f"psU_{fc}_{th}")
                psV = psB.tile([P, TH], f32, tag="psA", name=f"psV_{fc}_{th}")
                for q in range(2):
                    sl = slice(t0 + q * 512, t0 + (q + 1) * 512)
                    qsl = slice(q * 512, (q + 1) * 512)
                    for e2 in range(NE // 2):
                        nc.tensor.matmul(
                            psU[:, qsl],
                            w1_t[4 + e2][:, :, fc * P:(fc + 1) * P],
                            ht[:, 2 * e2:2 * e2 + 2, sl],
                            start=(e2 == 0), stop=(e2 == NE // 2 - 1),
                            perf_mode=mybir.MatmulPerfMode.DoubleRow,
                        )
                for q in range(2):
                    sl = slice(t0 + q * 512, t0 + (q + 1) * 512)
                    qsl = slice(q * 512, (q + 1) * 512)
                    for e2 in range(NE // 2):
                        nc.tensor.matmul(
                            psV[:, qsl],
                            w1_t[e2][:, :, fc * P:(fc + 1) * P],
                            ht[:, 2 * e2:2 * e2 + 2, sl],
                            start=(e2 == 0), stop=(e2 == NE // 2 - 1),
                            perf_mode=mybir.MatmulPerfMode.DoubleRow,
                        )
                cu = cupool.tile([P, TH], mybir.dt.bfloat16, tag="cu",
                                 name=f"cu_{fc}_{th}")
                init = carry[:, fc:fc + 1] if th == 0 else cu_last[:, fc:fc + 1]
                nc.vector.tensor_tensor_scan(
                    cu[:], psU[:], zeros[:], init, op0=ADD, op1=ADD
                )
                if th == 0:
                    nc.vector.tensor_copy(cu_last[:, fc:fc + 1], cu[:, TH - 1:TH])
                sl = slice(t0, t0 + TH)
                t1 = tmp.tile([P, TH], mybir.dt.bfloat16, tag="t1", name="t1")
                nc.vector.tensor_mul(t1[:], cu[:], bab[:, sl])
                wv = tmp.tile([P, TH], mybir.dt.bfloat16, tag="wv", name="wv")
                nc.vector.tensor_mul(wv[:], psU[:], bb[:, sl])
                z2 = tmp.tile([P, TH], mybir.dt.bfloat16, tag="z2", name="z2")
                nc.vector.scalar_tensor_tensor(
                    z2[:], a_bc[:, sl], ucol[:, fc:fc + 1], t1[:],
                    op0=mybir.AluOpType.mult, op1=mybir.AluOpType.add,
                )
                zv = tmp.tile([P, TH], mybir.dt.bfloat16, tag="zv", name="zv")
                nc.vector.tensor_add(zv[:], psV[:], z2[:])
                p1 = tmp.tile([P, TH], mybir.dt.bfloat16, tag="p1", name="p1")
                nc.vector.tensor_sub(p1[:], zv[:], wv[:])
                nc.scalar.activation(
                    x1[:, fc, sl], p1[:], ACT_FN, bias=b1c[:, fc:fc + 1],
                    scale=float(1.0 / 32.0),
                )

        # ---- phase 2+3 per token half ----
        for th in range(2):
            t0 = th * TH
            for fc2 in range(NF2):
                psX = psA.tile([P, TH], f32, tag="psA", name=f"psX_{fc2}_{th}")
                for q in range(2):
                    sl = slice(t0 + q * 512, t0 + (q + 1) * 512)
                    qsl = slice(q * 512, (q + 1) * 512)
                    for r2 in range(NF1 // 2):
                        nc.tensor.matmul(
                            psX[:, qsl],
                            w2_t[r2][:, :, fc2 * P:(fc2 + 1) * P],
                            x1[:, 2 * r2:2 * r2 + 2, sl],
                            start=(r2 == 0), stop=(r2 == NF1 // 2 - 1),
                            perf_mode=mybir.MatmulPerfMode.DoubleRow,
                        )
                nc.scalar.activation(
                    x2[:, fc2, t0:t0 + TH], psX[:], ACT_FN,
                    bias=b2c[:, fc2:fc2 + 1], scale=float(1.0 / 32.0),
                )
            for q in range(2):
                sl = slice(t0 + q * 512, t0 + (q + 1) * 512)
                psN = psB.tile([1, 512], f32, tag="psA", name=f"psN_{th}_{q}")
                for kc in range(NF2):
                    nc.tensor.matmul(
                        psN[:], w3_t[:, kc:kc + 1], x2[:, kc, sl],
                        start=(kc == 0), stop=(kc == NF2 - 1),
                    )
                nc.vector.tensor_copy(nf[:, sl], psN[:])
                nc.sync.dma_start(
                    out_d.rearrange("(a b) -> a b", a=1)[:, sl], nf[:, sl]
                )

    nc.compile()
    return nc


def _get_compiled():
    global _COMPILED
    if _COMPILED is None:
        _COMPILED = _build_nc()
    return _COMPILED


def _make_in_maps(inputs):
    bf = ml_dtypes.bfloat16
    h = np.ascontiguousarray(np.asarray(inputs["hidden_states"], dtype=np.float32))
    W1 = np.asarray(inputs["W1"], dtype=np.float32)
    W2 = np.asarray(inputs["W2"], dtype=np.float32)
    W3 = np.asarray(inputs["W3"], dtype=np.float32)
    b1 = np.asarray(inputs["b1"], dtype=np.float32)
    b2 = np.asarray(inputs["b2"], dtype=np.float32)
    b3 = np.asarray(inputs["b3"], dtype=np.float32)

    i = np.arange(S, dtype=np.float64)
    A = np.where(i < S - 1, 0.5 / np.maximum(S - 1 - i, 1), 0.0).astype(np.float32)
    Bv = np.where(i > 0, 0.5 / np.maximum(i, 1), 0.0).astype(np.float32)
    BA = (Bv - A).astype(np.float32)

    W1b = W1[E:]
    w1_f8 = np.ascontiguousarray((W1 * np.float32(32.0)).astype(ml_dtypes.float8_e4m3))
    w2_f8 = np.ascontiguousarray((W2 * np.float32(32.0)).astype(ml_dtypes.float8_e4m3))
    w3r = np.ascontiguousarray(W3[:, 0].reshape(NF2, P).T.astype(bf))
    b1c = np.ascontiguousarray(b1.reshape(NF1, P).T)
    b2c = np.ascontiguousarray(b2.reshape(NF2, P).T)

    in_maps = []
    for c in range(8):
        bi, half = divmod(c, 2)
        sl = slice(half * T, (half + 1) * T)
        U = (h[bi].sum(0, dtype=np.float64) @ W1b.astype(np.float64)).astype(np.float32)
        if half == 0:
            Bc = np.zeros(E, np.float32)
        else:
            Bc = (h[bi, :T].sum(0, dtype=np.float64) @ W1b.astype(np.float64)).astype(np.float32)
        in_maps.append({
            "h": np.ascontiguousarray(h[bi, sl].astype(bf)),
            "w1": w1_f8,
            "w2": w2_f8,
            "w3r": w3r,
            "ucol": np.ascontiguousarray((U * np.float32(32.0)).reshape(NF1, P).T),
            "b1c": b1c,
            "b2c": b2c,
            "carry": np.ascontiguousarray((Bc * np.float32(32.0)).reshape(NF1, P).T),
            "bab": np.ascontiguousarray(np.broadcast_to(BA[sl].astype(bf), (P, T))),
            "bb": np.ascontiguousarray(np.broadcast_to(Bv[sl].astype(bf), (P, T))),
        })
    return in_maps


def _finish(logits, inputs):
    b3 = np.asarray(inputs["b3"], dtype=np.float32)
    nf = np.float32(1.0) / (np.float32(1.0) + np.exp(-(logits + b3[0])))
    gt = np.float32(np.asarray(inputs["global_timestep"]))
    mask = np.asarray(inputs["token_mask"])
    ad = gt * (np.float32(0.5) + nf.astype(np.float32))
    ad = ad * (np.float32(1.0) + mask.astype(np.float32) * np.float32(0.3))
    ad = np.clip(ad, np.float32(0.0), np.float32(NUM_TIMESTEPS - 1))
    return ad.astype(np.int32)


def kernel(**inputs):
    from concourse import bass_utils

    nc = _get_compiled()
    in_maps = _make_in_maps(inputs)
    res = bass_utils.run_bass_kernel_spmd(nc, in_maps, core_ids=list(range(8)))
    nf = np.zeros((B, S), np.float32)
    for c in range(8):
        bi, half = divmod(c, 2)
        nf[bi, half * T:(half + 1) * T] = res.results[c]["out"]
    return _finish(nf, inputs)


# revision 37
# speedup vs baseline: 1.3817x; 1.1859x over previous
"""Trainium2 Bass kernel for nn_AdaptiveNoiseScheduler (segment_reduce).

Distribution: 8 NeuronCores = 4 batches x 2 sequence-halves, 2048 tokens/core,
MLP weights replicated (host pre-cast to bf16). The context term is rewritten
so the sequence cumsum happens AFTER the W1b projection (cumsum commutes with
the matmul), letting each core run fp32 free-axis DVE scans over its projected
activations. Cross-shard scan carries are 1024-float vectors: host folds them
into the scan initial state (bwd carry) and a rank-1 PSUM term (whole-batch
total), so no collectives are needed.

Layers 1 and 2 run fp8-e4m3 DoubleRow matmuls (weights host-prescaled by 32
into e4m3's normal range; the x32 factor flows consistently through the fp32
scan/combine chain — carries and U are host-scaled too — and unwinds in the
gelu activation's scale port). h is transposed on the PE in bf16 and cast to
fp8 in the PSUM->SBUF copy (fp8 PE-transpose needs stride-2 outputs, so the
transpose itself stays bf16). Measured output error vs the fp32 reference:
4.4e-3 norm relative (gate 2e-2).

Per-token pre-activation of layer 1:
    pre1[t] = h[t] @ W1a + b1 + A_t * U_tot + (B_t - A_t) * cu[t] - B_t * u[t]
where u = h @ W1b, cu = carry + inclusive-cumsum(u), A_t = 0.5/(S-1-t) gated,
B_t = 0.5/t gated. Everything runs in a transposed (feature-partition,
token-free) layout; biases ride the scalar-engine activation bias port; the
tiny elementwise finish (timestep scale, mask boost, clip, int cast) runs on
host over the (4,4096) result.
"""

from contextlib import ExitStack

import numpy as np
import ml_dtypes

P = 128
B, S, E = 4, 4096, 1024
T = S // 2          # tokens per core
TH = 1024           # tokens per PSUM stage (2 per core)
F1, F2 = 1024, 512
NE, NF1, NF2 = E // P, F1 // P, F2 // P
NUM_TIMESTEPS = 1000

_COMPILED = None


def _build_nc(act="Gelu"):
    import concourse.mybir as mybir
    import concourse.tile as tile
    from concourse import bacc
    from concourse.masks import make_identity

    f32, bf16 = mybir.dt.float32, mybir.dt.bfloat16
    AF = mybir.ActivationFunctionType
    ACT_FN = getattr(AF, act)
    ADD = mybir.AluOpType.add

    nc = bacc.Bacc("TRN2", target_bir_lowering=False, debug=False, num_devices=8)

    h_d = nc.dram_tensor("h", (T, E), bf16, kind="ExternalInput").ap()
    w1_d = nc.dram_tensor("w1", (2 * E, F1), mybir.dt.float8e4, kind="ExternalInput").ap()
    w2_d = nc.dram_tensor("w2", (F1, F2), mybir.dt.float8e4, kind="ExternalInput").ap()
    w3_d = nc.dram_tensor("w3r", (P, NF2), bf16, kind="ExternalInput").ap()
    uc_d = nc.dram_tensor("ucol", (P, NF1), f32, kind="ExternalInput").ap()
    b1c_d = nc.dram_tensor("b1c", (P, NF1), f32, kind="ExternalInput").ap()
    b2c_d = nc.dram_tensor("b2c", (P, NF2), f32, kind="ExternalInput").ap()
    car_d = nc.dram_tensor("carry", (P, NF1), f32, kind="ExternalInput").ap()
    bab_d = nc.dram_tensor("bab", (P, T), bf16, kind="ExternalInput").ap()
    bb_d = nc.dram_tensor("bb", (P, T), bf16, kind="ExternalInput").ap()
    out_d = nc.dram_tensor("out", (T,), f32, kind="ExternalOutput").ap()

    with tile.TileContext(nc) as tc, ExitStack() as ctx:
        const = ctx.enter_context(tc.tile_pool(name="const", bufs=1))
        hstage = ctx.enter_context(tc.tile_pool(name="hstage", bufs=2))
        big = ctx.enter_context(tc.tile_pool(name="big", bufs=1))
        cupool = ctx.enter_context(tc.tile_pool(name="cupool", bufs=3))
        tmp = ctx.enter_context(tc.tile_pool(name="tmp", bufs=2))
        psA = ctx.enter_context(tc.tile_pool(name="psA", bufs=4, space="PSUM"))
        psB = psA

        # ---- identity for PE transposes, built on the (idle) gpsimd engine
        ident = const.tile([P, P], bf16, name="ident")
        make_identity(nc, ident[:])

        # ---- h loads (HWDGE bf16), interleaved with W1 by first use ----
        hrows = [None] * 16
        w1_t = [None] * 8

        def load_hrow(r):
            t = hstage.tile([P, E], bf16, tag="hrow", name=f"hrow_{r}", bufs=16)
            nc.sync.dma_start(t[:], h_d[r * P:(r + 1) * P, :])
            hrows[r] = t

        def load_w1(r2):
            t = const.tile([P, 2, F1], mybir.dt.float8e4, name=f"w1_{r2}")
            nc.sync.dma_start(
                t[:],
                w1_d[2 * r2 * P:(2 * r2 + 2) * P, :].rearrange(
                    "(g p) f -> p g f", p=P
                ),
            )
            w1_t[r2] = t

        for r in range(8):          # first token half of h
            load_hrow(r)
        for r2 in range(4, 8):      # W1b (u-matmuls come first)
            load_w1(r2)

        zeros = const.tile([P, TH], bf16, name="zeros")
        nc.gpsimd.memset(zeros[:], 0.0)

        # ---- aux: tiny tensors first, coefficient rows split per token half
        ucol = const.tile([P, NF1], f32, name="ucol_t")
        nc.sync.dma_start(ucol[:], uc_d[:, :])
        carry = const.tile([P, NF1], f32, name="carry_t")
        nc.sync.dma_start(carry[:], car_d[:, :])
        b1c = const.tile([P, NF1], f32, name="b1c")
        nc.sync.dma_start(b1c[:], b1c_d[:, :])
        w3_t = const.tile([P, NF2], bf16, name="w3_t")
        nc.sync.dma_start(w3_t[:], w3_d[:, :])
        b2c = const.tile([P, NF2], f32, name="b2c")
        nc.sync.dma_start(b2c[:], b2c_d[:, :])
        bab = const.tile([P, T], bf16, name="bab_t")
        bb = const.tile([P, T], bf16, name="bb_t")
        a_bc = const.tile([P, T], bf16, name="a_bc")
        nc.sync.dma_start(bab[:, :TH], bab_d[:, :TH])
        nc.sync.dma_start(bb[:, :TH], bb_d[:, :TH])
        nc.vector.tensor_sub(a_bc[:, :TH], bb[:, :TH], bab[:, :TH])
        for r2 in range(4):         # W1a (needed from the first fc's psV)
            load_w1(r2)
        for r in range(8, 16):      # second token half of h (needed ~70us in)
            load_hrow(r)
        nc.sync.dma_start(bab[:, TH:], bab_d[:, TH:])
        nc.sync.dma_start(bb[:, TH:], bb_d[:, TH:])
        nc.vector.tensor_sub(a_bc[:, TH:], bb[:, TH:], bab[:, TH:])
        w2_t = []
        for r in range(NF1 // 2):
            t = const.tile([P, 2, F2], mybir.dt.float8e4, name=f"w2_{r}")
            nc.sync.dma_start(
                t[:],
                w2_d[2 * r * P:(2 * r + 2) * P, :].rearrange("(g p) f -> p g f", p=P),
            )
            w2_t.append(t)

        ht = big.tile([P, NE, T], mybir.dt.float8e4, name="ht")
        x1 = big.tile([P, NF1, T], mybir.dt.float8e4, name="x1")
        x2 = big.tile([P, NF2, T], bf16, name="x2")
        nf = big.tile([1, T], f32, name="nf")
        cu_last = big.tile([P, NF1], bf16, name="cu_last")

        # ---- phase 0: PE-transpose h rows into ht, two rows per PSUM tile.
        #      Upfront copies ride DVE (idle then); in-sweep ones ride ACT.
        def transpose_rows(r0, eng="vector"):
            pst = psA.tile([P, 2 * TH], bf16, tag="psA", name=f"pst_{r0}")
            for g in range(2):
                for ec in range(NE):
                    nc.tensor.transpose(
                        pst[:, g * TH + ec * P:g * TH + (ec + 1) * P],
                        hrows[r0 + g][:, ec * P:(ec + 1) * P],
                        ident[:],
                    )
            dst = ht[:, :, r0 * P:(r0 + 2) * P].rearrange("p a (g b) -> p g a b", g=2)
            src = pst[:].rearrange("p (g a b) -> p g a b", g=2, b=P)
            if eng == "vector":
                nc.vector.tensor_copy(dst, src)
            else:
                nc.scalar.copy(dst, src)

        for r0 in range(0, 8, 2):
            transpose_rows(r0, eng="scalar")

        # ---- phase 1 (th-major; combine is software-pipelined one fc behind
        #      the matmul/scan front so the in-order DVE stream never blocks
        #      on the gpsimd products)
        def make_combine(cu, psV, fc, t0, init):
            def emit():
                sl = slice(t0, t0 + TH)
                t2 = tmp.tile([P, TH], mybir.dt.bfloat16, tag="t2", name="t2")
                nc.gpsimd.tensor_mul(t2[:, 0:1], init, bb[:, t0:t0 + 1])
                nc.gpsimd.tensor_mul(
                    t2[:, 1:], cu[:, :TH - 1], bb[:, t0 + 1:t0 + TH]
                )
                t3 = tmp.tile([P, TH], mybir.dt.bfloat16, tag="t3", name="t3")
                nc.gpsimd.tensor_mul(t3[:], cu[:], a_bc[:, sl])
                z4 = tmp.tile([P, TH], mybir.dt.bfloat16, tag="z4", name="z4")
                nc.vector.scalar_tensor_tensor(
                    z4[:], a_bc[:, sl], ucol[:, fc:fc + 1], t3[:],
                    op0=mybir.AluOpType.mult, op1=mybir.AluOpType.subtract,
                )
                zv = tmp.tile([P, TH], mybir.dt.bfloat16, tag="zv", name="zv")
                nc.vector.tensor_add(zv[:], psV[:], z4[:])
                p1 = tmp.tile([P, TH], mybir.dt.bfloat16, tag="p1", name="p1")
                nc.vector.tensor_add(p1[:], zv[:], t2[:])
                nc.scalar.activation(
                    x1[:, fc, sl], p1[:], ACT_FN, bias=b1c[:, fc:fc + 1],
                    scale=float(1.0 / 32.0),
                )
            return emit

        pending = None
        for th in range(2):
            t0 = th * TH
            for fc in range(NF1):
                if th == 0 and fc in (2, 3, 4, 5):
                    transpose_rows(8 + 2 * (fc - 2), eng="scalar")
                psU = psA.tile([P, TH], f32, tag="psA", name=f"psU_{fc}_{th}")
                psV = psB.tile([P, TH], f32, tag="psA", name=f"psV_{fc}_{th}")
                for q in range(2):
                    sl = slice(t0 + q * 512, t0 + (q + 1) * 512)
                    qsl = slice(q * 512, (q + 1) * 512)
                    for e2 in range(NE // 2):
                        nc.tensor.matmul(
                            psU[:, qsl],
                            w1_t[4 + e2][:, :, fc * P:(fc + 1) * P],
                            ht[:, 2 * e2:2 * e2 + 2, sl],
                            start=(e2 == 0), stop=(e2 == NE // 2 - 1),
                            perf_mode=mybir.MatmulPerfMode.DoubleRow,
                        )
                for q in range(2):
                    sl = slice(t0 + q * 512, t0 + (q + 1) * 512)
                    qsl = slice(q * 512, (q + 1) * 512)
                    for e2 in range(NE // 2):
                        nc.tensor.matmul(
                            psV[:, qsl],
                            w1_t[e2][:, :, fc * P:(fc + 1) * P],
                            ht[:, 2 * e2:2 * e2 + 2, sl],
                            start=(e2 == 0), stop=(e2 == NE // 2 - 1),
                            perf_mode=mybir.MatmulPerfMode.DoubleRow,
                        )
                cu = cupool.tile([P, TH], mybir.dt.bfloat16, tag="cu",
                                 name=f"cu_{fc}_{th}")
                init = carry[:, fc:fc + 1] if th == 0 else cu_last[:, fc:fc + 1]
                nc.vector.tensor_tensor_scan(
                    cu[:], psU[:], zeros[:], init, op0=ADD, op1=ADD
                )
                if th == 0:
                    nc.vector.tensor_copy(cu_last[:, fc:fc + 1], cu[:, TH - 1:TH])
                if pending is not None:
                    pending()
                pending = make_combine(cu, psV, fc, t0, init)
        pending()

        # ---- phase 2+3 per token half ----
        for th in range(2):
            t0 = th * TH
            for fc2 in range(NF2):
                psX = psA.tile([P, TH], f32, tag="psA", name=f"psX_{fc2}_{th}")
                for q in range(2):
                    sl = slice(t0 + q * 512, t0 + (q + 1) * 512)
                    qsl = slice(q * 512, (q + 1) * 512)
                    for r2 in range(NF1 // 2):
                        nc.tensor.matmul(
                            psX[:, qsl],
                            w2_t[r2][:, :, fc2 * P:(fc2 + 1) * P],
                            x1[:, 2 * r2:2 * r2 + 2, sl],
                            start=(r2 == 0), stop=(r2 == NF1 // 2 - 1),
                            perf_mode=mybir.MatmulPerfMode.DoubleRow,
                        )
                nc.scalar.activation(
                    x2[:, fc2, t0:t0 + TH], psX[:], ACT_FN,
                    bias=b2c[:, fc2:fc2 + 1], scale=float(1.0 / 32.0),
                )
            for q in range(2):
                sl = slice(t0 + q * 512, t0 + (q + 1) * 512)
                psN = psB.tile([1, 512], f32, tag="psA", name=f"psN_{th}_{q}")
                for kc in range(NF2):
                    nc.tensor.matmul(
                        psN[:], w3_t[:, kc:kc + 1], x2[:, kc, sl],
                        start=(kc == 0), stop=(kc == NF2 - 1),
                    )
                nc.vector.tensor_copy(nf[:, sl], psN[:])
                nc.sync.dma_start(
                    out_d.rearrange("(a b) -> a b", a=1)[:, sl], nf[:, sl]
                )

    nc.compile()
    return nc


def _get_compiled():
    global _COMPILED
    if _COMPILED is None:
        _COMPILED = _build_nc()
    return _COMPILED


def _make_in_maps(inputs):
    bf = ml_dtypes.bfloat16
    h = np.ascontiguousarray(np.asarray(inputs["hidden_states"], dtype=np.float32))
    W1 = np.asarray(inputs["W1"], dtype=np.float32)
    W2 = np.asarray(inputs["W2"], dtype=np.float32)
    W3 = np.asarray(inputs["W3"], dtype=np.float32)
    b1 = np.asarray(inputs["b1"], dtype=np.float32)
    b2 = np.asarray(inputs["b2"], dtype=np.float32)
    b3 = np.asarray(inputs["b3"], dtype=np.float32)

    i = np.arange(S, dtype=np.float64)
    A = np.where(i < S - 1, 0.5 / np.maximum(S - 1 - i, 1), 0.0).astype(np.float32)
    Bv = np.where(i > 0, 0.5 / np.maximum(i, 1), 0.0).astype(np.float32)
    BA = (Bv - A).astype(np.float32)

    W1b = W1[E:]
    w1_f8 = np.ascontiguousarray((W1 * np.float32(32.0)).astype(ml_dtypes.float8_e4m3))
    w2_f8 = np.ascontiguousarray((W2 * np.float32(32.0)).astype(ml_dtypes.float8_e4m3))
    w3r = np.ascontiguousarray(W3[:, 0].reshape(NF2, P).T.astype(bf))
    b1c = np.ascontiguousarray(b1.reshape(NF1, P).T)
    b2c = np.ascontiguousarray(b2.reshape(NF2, P).T)

    in_maps = []
    for c in range(8):
        bi, half = divmod(c, 2)
        sl = slice(half * T, (half + 1) * T)
        U = (h[bi].sum(0, dtype=np.float64) @ W1b.astype(np.float64)).astype(np.float32)
        if half == 0:
            Bc = np.zeros(E, np.float32)
        else:
            Bc = (h[bi, :T].sum(0, dtype=np.float64) @ W1b.astype(np.float64)).astype(np.float32)
        in_maps.append({
            "h": np.ascontiguousarray(h[bi, sl].astype(bf)),
            "w1": w1_f8,
            "w2": w2_f8,
            "w3r": w3r,
            "ucol": np.ascontiguousarray((U * np.float32(32.0)).reshape(NF1, P).T),
            "b1c": b1c,
            "b2c": b2c,
            "carry": np.ascontiguousarray((Bc * np.float32(32.0)).reshape(NF1, P).T),
            "bab": np.ascontiguousarray(np.broadcast_to(BA[sl].astype(bf), (P, T))),
            "bb": np.ascontiguousarray(np.broadcast_to(Bv[sl].astype(bf), (P, T))),
        })
    return in_maps


def _finish(logits, inputs):
    b3 = np.asarray(inputs["b3"], dtype=np.float32)
    nf = np.float32(1.0) / (np.float32(1.0) + np.exp(-(logits + b3[0])))
    gt = np.float32(np.asarray(inputs["global_timestep"]))
    mask = np.asarray(inputs["token_mask"])
    ad = gt * (np.float32(0.5) + nf.astype(np.float32))
    ad = ad * (np.float32(1.0) + mask.astype(np.float32) * np.float32(0.3))
    ad = np.clip(ad, np.float32(0.0), np.float32(NUM_TIMESTEPS - 1))
    return ad.astype(np.int32)


def kernel(**inputs):
    from concourse import bass_utils

    nc = _get_compiled()
    in_maps = _make_in_maps(inputs)
    res = bass_utils.run_bass_kernel_spmd(nc, in_maps, core_ids=list(range(8)))
    nf = np.zeros((B, S), np.float32)
    for c in range(8):
        bi, half = divmod(c, 2)
        nf[bi, half * T:(half + 1) * T] = res.results[c]["out"]
    return _finish(nf, inputs)


# revision 41
# speedup vs baseline: 1.6455x; 1.1909x over previous
"""Trainium2 Bass kernel for nn_AdaptiveNoiseScheduler (segment_reduce).

Distribution: 8 NeuronCores = 4 batches x 2 sequence-halves, 2048 tokens/core,
MLP weights replicated (host pre-cast to bf16). The context term is rewritten
so the sequence cumsum happens AFTER the W1b projection (cumsum commutes with
the matmul), letting each core run fp32 free-axis DVE scans over its projected
activations. Cross-shard scan carries are 1024-float vectors: host folds them
into the scan initial state (bwd carry) and a rank-1 PSUM term (whole-batch
total), so no collectives are needed.

Layers 1 and 2 run fp8-e4m3 DoubleRow matmuls (weights host-prescaled by 32
into e4m3's normal range; the x32 factor flows consistently through the fp32
scan/combine chain — carries and U are host-scaled too — and unwinds in the
gelu activation's scale port). h is transposed on the PE in bf16 and cast to
fp8 in the PSUM->SBUF copy (fp8 PE-transpose needs stride-2 outputs, so the
transpose itself stays bf16). Measured output error vs the fp32 reference:
4.4e-3 norm relative (gate 2e-2).

Per-token pre-activation of layer 1:
    pre1[t] = h[t] @ W1a + b1 + A_t * U_tot + (B_t - A_t) * cu[t] - B_t * u[t]
where u = h @ W1b, cu = carry + inclusive-cumsum(u), A_t = 0.5/(S-1-t) gated,
B_t = 0.5/t gated. Everything runs in a transposed (feature-partition,
token-free) layout; biases ride the scalar-engine activation bias port; the
tiny elementwise finish (timestep scale, mask boost, clip, int cast) runs on
host over the (4,4096) result.
"""

from contextlib import ExitStack

import numpy as np
import ml_dtypes

P = 128
B, S, E = 4, 4096, 1024
T = S // 2          # tokens per core
TH = 1024           # tokens per PSUM stage (2 per core)
F1, F2 = 1024, 512
NE, NF1, NF2 = E // P, F1 // P, F2 // P
NUM_TIMESTEPS = 1000

_COMPILED = None


def _build_nc(act="Gelu"):
    import concourse.mybir as mybir
    import concourse.tile as tile
    from concourse import bacc
    from concourse.masks import make_identity

    f32, bf16 = mybir.dt.float32, mybir.dt.bfloat16
    AF = mybir.ActivationFunctionType
    ACT_FN = getattr(AF, act)
    ADD = mybir.AluOpType.add

    nc = bacc.Bacc("TRN2", target_bir_lowering=False, debug=False, num_devices=8)

    h_d = nc.dram_tensor("h", (T, E), bf16, kind="ExternalInput").ap()
    w1_d = nc.dram_tensor("w1", (2 * E, F1), mybir.dt.float8e4, kind="ExternalInput").ap()
    w2_d = nc.dram_tensor("w2", (F1, F2), mybir.dt.float8e4, kind="ExternalInput").ap()
    w3_d = nc.dram_tensor("w3r", (P, NF2), bf16, kind="ExternalInput").ap()
    rk1_d = nc.dram_tensor("rk1", (1, F1), bf16, kind="ExternalInput").ap()
    rkr_d = nc.dram_tensor("rkr", (1, T), bf16, kind="ExternalInput").ap()
    b1c_d = nc.dram_tensor("b1c", (P, NF1), f32, kind="ExternalInput").ap()
    b2c_d = nc.dram_tensor("b2c", (P, NF2), f32, kind="ExternalInput").ap()
    car_d = nc.dram_tensor("carry", (P, NF1), f32, kind="ExternalInput").ap()
    bab_d = nc.dram_tensor("bab", (P, T), bf16, kind="ExternalInput").ap()
    bb_d = nc.dram_tensor("bb", (P, T), bf16, kind="ExternalInput").ap()
    out_d = nc.dram_tensor("out", (T,), f32, kind="ExternalOutput").ap()

    with tile.TileContext(nc) as tc, ExitStack() as ctx:
        const = ctx.enter_context(tc.tile_pool(name="const", bufs=1))
        hstage = ctx.enter_context(tc.tile_pool(name="hstage", bufs=2))
        big = ctx.enter_context(tc.tile_pool(name="big", bufs=1))
        cupool = ctx.enter_context(tc.tile_pool(name="cupool", bufs=3))
        tmp = ctx.enter_context(tc.tile_pool(name="tmp", bufs=2))
        psA = ctx.enter_context(tc.tile_pool(name="psA", bufs=4, space="PSUM"))
        psB = psA

        # ---- identity for PE transposes, built on the (idle) gpsimd engine
        ident = const.tile([P, P], bf16, name="ident")
        make_identity(nc, ident[:])

        # ---- h loads (HWDGE bf16), interleaved with W1 by first use ----
        hrows = [None] * 16
        w1_t = [None] * 8

        def load_hrow(r):
            t = hstage.tile([P, E], bf16, tag="hrow", name=f"hrow_{r}", bufs=16)
            nc.sync.dma_start(t[:], h_d[r * P:(r + 1) * P, :])
            hrows[r] = t

        def load_w1(r2):
            t = const.tile([P, 2, F1], mybir.dt.float8e4, name=f"w1_{r2}")
            nc.sync.dma_start(
                t[:],
                w1_d[2 * r2 * P:(2 * r2 + 2) * P, :].rearrange(
                    "(g p) f -> p g f", p=P
                ),
            )
            w1_t[r2] = t

        for r in range(8):          # first token half of h
            load_hrow(r)
        for r2 in range(4, 8):      # W1b (u-matmuls come first)
            load_w1(r2)

        zeros = const.tile([P, TH], bf16, name="zeros")
        nc.gpsimd.memset(zeros[:], 0.0)

        # ---- aux: tiny tensors first, coefficient rows split per token half
        rk1_t = const.tile([1, F1], bf16, name="rk1_t")
        nc.sync.dma_start(rk1_t[:], rk1_d[:, :])
        rkr_t = const.tile([1, T], bf16, name="rkr_t")
        nc.sync.dma_start(rkr_t[:], rkr_d[:, :])
        carry = const.tile([P, NF1], f32, name="carry_t")
        nc.sync.dma_start(carry[:], car_d[:, :])
        b1c = const.tile([P, NF1], f32, name="b1c")
        nc.sync.dma_start(b1c[:], b1c_d[:, :])
        w3_t = const.tile([P, NF2], bf16, name="w3_t")
        nc.sync.dma_start(w3_t[:], w3_d[:, :])
        b2c = const.tile([P, NF2], f32, name="b2c")
        nc.sync.dma_start(b2c[:], b2c_d[:, :])
        bab = const.tile([P, T], bf16, name="bab_t")
        bb = const.tile([P, T], bf16, name="bb_t")
        a_bc = const.tile([P, T], bf16, name="a_bc")
        nc.sync.dma_start(bab[:, :TH], bab_d[:, :TH])
        nc.sync.dma_start(bb[:, :TH], bb_d[:, :TH])
        nc.vector.tensor_sub(a_bc[:, :TH], bb[:, :TH], bab[:, :TH])
        for r2 in range(4):         # W1a (needed from the first fc's psV)
            load_w1(r2)
        for r in range(8, 16):      # second token half of h (needed ~70us in)
            load_hrow(r)
        nc.sync.dma_start(bab[:, TH:], bab_d[:, TH:])
        nc.sync.dma_start(bb[:, TH:], bb_d[:, TH:])
        nc.vector.tensor_sub(a_bc[:, TH:], bb[:, TH:], bab[:, TH:])
        w2_t = []
        for r in range(NF1 // 2):
            t = const.tile([P, 2, F2], mybir.dt.float8e4, name=f"w2_{r}")
            nc.sync.dma_start(
                t[:],
                w2_d[2 * r * P:(2 * r + 2) * P, :].rearrange("(g p) f -> p g f", p=P),
            )
            w2_t.append(t)

        ht = big.tile([P, NE, T], mybir.dt.float8e4, name="ht")
        x1 = big.tile([P, NF1, T], mybir.dt.float8e4, name="x1")
        x2 = big.tile([P, NF2, T], bf16, name="x2")
        nf = big.tile([1, T], f32, name="nf")
        cu_last = big.tile([P, NF1], bf16, name="cu_last")

        # ---- phase 0: PE-transpose h rows into ht, two rows per PSUM tile.
        #      Upfront copies ride DVE (idle then); in-sweep ones ride ACT.
        def transpose_rows(r0, eng="vector"):
            pst = psA.tile([P, 2 * TH], bf16, tag="psA", name=f"pst_{r0}")
            for g in range(2):
                for ec in range(NE):
                    nc.tensor.transpose(
                        pst[:, g * TH + ec * P:g * TH + (ec + 1) * P],
                        hrows[r0 + g][:, ec * P:(ec + 1) * P],
                        ident[:],
                    )
            dst = ht[:, :, r0 * P:(r0 + 2) * P].rearrange("p a (g b) -> p g a b", g=2)
            src = pst[:].rearrange("p (g a b) -> p g a b", g=2, b=P)
            if eng == "vector":
                nc.vector.tensor_copy(dst, src)
            else:
                nc.scalar.copy(dst, src)

        for r0 in range(0, 8, 2):
            transpose_rows(r0, eng="scalar")

        # ---- phase 1 (th-major; combine is software-pipelined one fc behind
        #      the matmul/scan front so the in-order DVE stream never blocks
        #      on the gpsimd products)
        def make_combine(cu, psV, fc, t0, init):
            def emit():
                sl = slice(t0, t0 + TH)
                t2 = tmp.tile([P, TH], mybir.dt.bfloat16, tag="t2", name="t2")
                nc.gpsimd.tensor_mul(t2[:, 0:1], init, bb[:, t0:t0 + 1])
                nc.gpsimd.tensor_mul(
                    t2[:, 1:], cu[:, :TH - 1], bb[:, t0 + 1:t0 + TH]
                )
                t3 = tmp.tile([P, TH], mybir.dt.bfloat16, tag="t3", name="t3")
                nc.vector.tensor_mul(t3[:], cu[:], a_bc[:, sl])
                zv = tmp.tile([P, TH], mybir.dt.bfloat16, tag="zv", name="zv")
                nc.vector.tensor_sub(zv[:], psV[:], t3[:])
                p1 = tmp.tile([P, TH], mybir.dt.bfloat16, tag="p1", name="p1")
                nc.vector.tensor_add(p1[:], zv[:], t2[:])
                nc.scalar.activation(
                    x1[:, fc, sl], p1[:], ACT_FN, bias=b1c[:, fc:fc + 1],
                    scale=float(1.0 / 32.0),
                )
            return emit

        pending = None
        for th in range(2):
            t0 = th * TH
            for fc in range(NF1):
                if th == 0 and fc in (2, 3, 4, 5):
                    transpose_rows(8 + 2 * (fc - 2), eng="scalar")
                psU = psA.tile([P, TH], f32, tag="psA", name=f"psU_{fc}_{th}")
                psV = psB.tile([P, TH], f32, tag="psA", name=f"psV_{fc}_{th}")
                for q in range(2):
                    sl = slice(t0 + q * 512, t0 + (q + 1) * 512)
                    qsl = slice(q * 512, (q + 1) * 512)
                    for e2 in range(NE // 2):
                        nc.tensor.matmul(
                            psU[:, qsl],
                            w1_t[4 + e2][:, :, fc * P:(fc + 1) * P],
                            ht[:, 2 * e2:2 * e2 + 2, sl],
                            start=(e2 == 0), stop=(e2 == NE // 2 - 1),
                            perf_mode=mybir.MatmulPerfMode.DoubleRow,
                        )
                for q in range(2):
                    sl = slice(t0 + q * 512, t0 + (q + 1) * 512)
                    qsl = slice(q * 512, (q + 1) * 512)
                    for e2 in range(NE // 2):
                        nc.tensor.matmul(
                            psV[:, qsl],
                            w1_t[e2][:, :, fc * P:(fc + 1) * P],
                            ht[:, 2 * e2:2 * e2 + 2, sl],
                            start=(e2 == 0), stop=(e2 == NE // 2 - 1),
                            perf_mode=mybir.MatmulPerfMode.DoubleRow,
                        )
                    nc.tensor.matmul(
                        psV[:, qsl], rk1_t[0:1, fc * P:(fc + 1) * P],
                        rkr_t[0:1, sl], start=False, stop=True,
                        skip_group_check=True,
                    )
                cu = cupool.tile([P, TH], mybir.dt.bfloat16, tag="cu",
                                 name=f"cu_{fc}_{th}")
                init = carry[:, fc:fc + 1] if th == 0 else cu_last[:, fc:fc + 1]
                nc.vector.tensor_tensor_scan(
                    cu[:], psU[:], zeros[:], init, op0=ADD, op1=ADD
                )
                if th == 0:
                    nc.vector.tensor_copy(cu_last[:, fc:fc + 1], cu[:, TH - 1:TH])
                if pending is not None:
                    pending()
                pending = make_combine(cu, psV, fc, t0, init)
        pending()

        # ---- phase 2+3 per token half ----
        for th in range(2):
            t0 = th * TH
            for fc2 in range(NF2):
                psX = psA.tile([P, TH], f32, tag="psA", name=f"psX_{fc2}_{th}")
                for q in range(2):
                    sl = slice(t0 + q * 512, t0 + (q + 1) * 512)
                    qsl = slice(q * 512, (q + 1) * 512)
                    for r2 in range(NF1 // 2):
                        nc.tensor.matmul(
                            psX[:, qsl],
                            w2_t[r2][:, :, fc2 * P:(fc2 + 1) * P],
                            x1[:, 2 * r2:2 * r2 + 2, sl],
                            start=(r2 == 0), stop=(r2 == NF1 // 2 - 1),
                            perf_mode=mybir.MatmulPerfMode.DoubleRow,
                        )
                nc.scalar.activation(
                    x2[:, fc2, t0:t0 + TH], psX[:], ACT_FN,
                    bias=b2c[:, fc2:fc2 + 1], scale=float(1.0 / 32.0),
                )
            for q in range(2):
                sl = slice(t0 + q * 512, t0 + (q + 1) * 512)
                psN = psB.tile([1, 512], f32, tag="psA", name=f"psN_{th}_{q}")
                for kc in range(NF2):
                    nc.tensor.matmul(
                        psN[:], w3_t[:, kc:kc + 1], x2[:, kc, sl],
                        start=(kc == 0), stop=(kc == NF2 - 1),
                    )
                nc.vector.tensor_copy(nf[:, sl], psN[:])
                nc.sync.dma_start(
                    out_d.rearrange("(a b) -> a b", a=1)[:, sl], nf[:, sl]
                )

    nc.compile()
    return nc


def _get_compiled():
    global _COMPILED
    if _COMPILED is None:
        _COMPILED = _build_nc()
    return _COMPILED


def _make_in_maps(inputs):
    bf = ml_dtypes.bfloat16
    h = np.ascontiguousarray(np.asarray(inputs["hidden_states"], dtype=np.float32))
    W1 = np.asarray(inputs["W1"], dtype=np.float32)
    W2 = np.asarray(inputs["W2"], dtype=np.float32)
    W3 = np.asarray(inputs["W3"], dtype=np.float32)
    b1 = np.asarray(inputs["b1"], dtype=np.float32)
    b2 = np.asarray(inputs["b2"], dtype=np.float32)
    b3 = np.asarray(inputs["b3"], dtype=np.float32)

    i = np.arange(S, dtype=np.float64)
    A = np.where(i < S - 1, 0.5 / np.maximum(S - 1 - i, 1), 0.0).astype(np.float32)
    Bv = np.where(i > 0, 0.5 / np.maximum(i, 1), 0.0).astype(np.float32)
    BA = (Bv - A).astype(np.float32)

    W1b = W1[E:]
    w1_f8 = np.ascontiguousarray((W1 * np.float32(32.0)).astype(ml_dtypes.float8_e4m3))
    w2_f8 = np.ascontiguousarray((W2 * np.float32(32.0)).astype(ml_dtypes.float8_e4m3))
    w3r = np.ascontiguousarray(W3[:, 0].reshape(NF2, P).T.astype(bf))
    b1c = np.ascontiguousarray(b1.reshape(NF1, P).T)
    b2c = np.ascontiguousarray(b2.reshape(NF2, P).T)

    in_maps = []
    for c in range(8):
        bi, half = divmod(c, 2)
        sl = slice(half * T, (half + 1) * T)
        U = (h[bi].sum(0, dtype=np.float64) @ W1b.astype(np.float64)).astype(np.float32)
        if half == 0:
            Bc = np.zeros(E, np.float32)
        else:
            Bc = (h[bi, :T].sum(0, dtype=np.float64) @ W1b.astype(np.float64)).astype(np.float32)
        in_maps.append({
            "h": np.ascontiguousarray(h[bi, sl].astype(bf)),
            "w1": w1_f8,
            "w2": w2_f8,
            "w3r": w3r,
            "rk1": np.ascontiguousarray((U * np.float32(32.0)).reshape(1, F1).astype(bf)),
            "rkr": np.ascontiguousarray(A[sl].reshape(1, T).astype(bf)),
            "b1c": b1c,
            "b2c": b2c,
            "carry": np.ascontiguousarray((Bc * np.float32(32.0)).reshape(NF1, P).T),
            "bab": np.ascontiguousarray(np.broadcast_to(BA[sl].astype(bf), (P, T))),
            "bb": np.ascontiguousarray(np.broadcast_to(Bv[sl].astype(bf), (P, T))),
        })
    return in_maps


def _finish(logits, inputs):
    b3 = np.asarray(inputs["b3"], dtype=np.float32)
    nf = np.float32(1.0) / (np.float32(1.0) + np.exp(-(logits + b3[0])))
    gt = np.float32(np.asarray(inputs["global_timestep"]))
    mask = np.asarray(inputs["token_mask"])
    ad = gt * (np.float32(0.5) + nf.astype(np.float32))
    ad = ad * (np.float32(1.0) + mask.astype(np.float32) * np.float32(0.3))
    ad = np.clip(ad, np.float32(0.0), np.float32(NUM_TIMESTEPS - 1))
    return ad.astype(np.int32)


def kernel(**inputs):
    from concourse import bass_utils

    nc = _get_compiled()
    in_maps = _make_in_maps(inputs)
    res = bass_utils.run_bass_kernel_spmd(nc, in_maps, core_ids=list(range(8)))
    nf = np.zeros((B, S), np.float32)
    for c in range(8):
        bi, half = divmod(c, 2)
        nf[bi, half * T:(half + 1) * T] = res.results[c]["out"]
    return _finish(nf, inputs)
